# revision 25
# baseline (speedup 1.0000x reference)
"""ContentPhasorStream kernel for 8 Trainium2 NeuronCores.

Math: the reference is causal linear attention with feature map
[cos(phase), sin(phase)] (2K = 64 features):

  retrieved[b,l,d] = sum_{t<=l} v[b,t,d] * sum_k cos(qp[b,l,k] - kp[b,t,k])
                   = Qf[l] @ cumsum_t(Kf[t]^T v[t])      (Qf/Kf = [cos, sin] feats)

so the [B,L,K,D] cumsum never needs to be materialized.  We run a chunked
scan (128-position chunks): per chunk O = mask(Qf Kf^T) V + Qf S with
S += Kf^T V.

Sharding: 8 cores = 2 batches x 4 L-segments of 512.  Phase A computes the
MLPs + intra-segment attention + the segment summary state S_seg [64,256]
per core.  The host then prefix-sums the tiny per-segment states (pure
elementwise adds) and phase B applies the cross-segment correction
out = (Qf @ S_prefix) * norm + O_intra_scaled on-device.

All matmul operand paths use float32r (fp32 bits in the 4x-faster PE
dataflow mode); accumulation stays fp32 in PSUM.

Perf structure (from NTFF traces):
 - inputs packed into 5 large DMAs, issued on two HWDGE rings (sync+scalar),
   first-needed weights first
 - scratch warmup matmuls ramp the PE clock (HAM) during the DMA wait
 - ACT runs tanh x6 then sin x2 (2 LUT loads, no thrash); |t| for the
   cos rows is done on DVE as a sign-bit mask
"""

import math
import sys

import numpy as np

for _p in ("/opt/trn_rl_repo", "/root/.axon_site/_ro/trn_rl_repo"):
    if _p not in sys.path:
        sys.path.insert(0, _p)

import concourse.bass as bass
import concourse.mybir as mybir
import concourse.tile as tile
from concourse import bacc
from concourse.bass_utils import run_bass_kernel_spmd
from concourse.masks import make_upper_triangular

B, L, D, NK = 2, 2048, 256, 32
NCORES = 8
NSEG = NCORES // B          # 4 segments per batch
SEG = L // NSEG             # 512 positions per core
CH = 128                    # attention chunk
NCH = SEG // CH             # 4 chunks per segment
F = 2 * NK                  # 64 = [cos, sin] feature dim
PI = math.pi
NWARM = 7                   # PE clock-ramp matmuls
NFILL = 6                   # PE keep-warm fillers over the feature-wait gap

F32 = mybir.dt.float32
FR = mybir.dt.float32r
U32 = mybir.dt.uint32
TANH = mybir.ActivationFunctionType.Tanh
SIN = mybir.ActivationFunctionType.Sin
PSUM = bass.MemorySpace.PSUM
AND = mybir.AluOpType.bitwise_and
MULT = mybir.AluOpType.mult
ADD = mybir.AluOpType.add

# cpf (fp32 constant pack) column layout
CPF_VB = 0            # [128, 256] broadcast value bias
CPF_B1K = 256         # [128, 2]
CPF_B1Q = 258         # [128, 2]
CPF_B2K = 260         # [64, 1] doubled key phase bias
CPF_B2Q = 261
CPF_PSC = 262         # [64, 1] sin scale  [-pi x32; +pi x32]
CPF_PBI = 263         # [64, 1] sin bias   [pi/2 x32; 0 x32]
CPF_NRM = 264         # [128, 4] per-chunk 1/sqrt((pos+1)K)
CPF_N = 268


def _build_phase_a():
    nc = bacc.Bacc("TRN2", target_bir_lowering=False, debug=False)

    w1k_d = nc.dram_tensor("w1k", [128, 2, D], FR, kind="ExternalInput").ap()
    xt_d = nc.dram_tensor("xt", [128, 2, SEG], FR, kind="ExternalInput").ap()
    wqv_d = nc.dram_tensor("wqv", [128, 4, D], FR, kind="ExternalInput").ap()
    cpr_d = nc.dram_tensor("cpr", [128, 5, F], FR, kind="ExternalInput").ap()
    cpf_d = nc.dram_tensor("cpf", [128, CPF_N], F32, kind="ExternalInput").ap()

    o_d = nc.dram_tensor("o", [SEG, D], F32, kind="ExternalOutput").ap()
    qft_d = nc.dram_tensor("qft", [F, SEG], FR, kind="ExternalOutput").ap()
    s_d = nc.dram_tensor("s", [F, D], FR, kind="ExternalOutput").ap()

    with tile.TileContext(nc) as tc:
        with (
            tc.tile_pool(name="const", bufs=1) as constp,
            tc.tile_pool(name="hsb", bufs=4) as hsbp,
            tc.tile_pool(name="work", bufs=2) as workp,
        ):
            # ---- PE warmup on scratch data (ramp HAM during DMA wait) ----
            warm_sb = constp.tile([128, SEG], FR)
            nc.vector.memset(warm_sb[:].bitcast(U32), 0)
            # preload the tanh LUT off the critical chain while DMAs are in
            # flight (each tanh<->sin switch reloads the single table slot,
            # so only the first function is worth preloading)
            lutw = constp.tile([1, 1], F32)
            nc.scalar.activation(lutw[:], warm_sb[0:1, 0:1].bitcast(F32), TANH)

            # ---- packed input DMAs, two HWDGE rings, first-needed first ----
            w1k_sb = constp.tile([128, 2, D], FR)
            nc.sync.dma_start(w1k_sb[:], w1k_d[:, :, :])
            xt_sb = constp.tile([128, 2, SEG], FR)
            nc.scalar.dma_start(xt_sb[:, 0, :], xt_d[:, 0, :])
            nc.sync.dma_start(xt_sb[:, 1, :], xt_d[:, 1, :])
            wqv_sb = constp.tile([128, 4, D], FR)
            nc.scalar.dma_start(wqv_sb[:], wqv_d[:, :, :])
            cpf_sb = constp.tile([128, CPF_N], F32)
            nc.sync.dma_start(cpf_sb[:], cpf_d[:, :])
            cpr_sb = constp.tile([128, 5, F], FR)
            nc.scalar.dma_start(cpr_sb[:], cpr_d[:, :, :])

            w1 = {"k": w1k_sb, "q": None}
            vb_ap = cpf_sb[:, CPF_VB:CPF_VB + D]
            b1_ap = {"k": cpf_sb[:, CPF_B1K:CPF_B1K + 2],
                     "q": cpf_sb[:, CPF_B1Q:CPF_B1Q + 2]}
            b2_ap = {"k": cpf_sb[0:F, CPF_B2K:CPF_B2K + 1],
                     "q": cpf_sb[0:F, CPF_B2Q:CPF_B2Q + 1]}
            psc_ap = cpf_sb[0:F, CPF_PSC:CPF_PSC + 1]
            pbi_ap = cpf_sb[0:F, CPF_PBI:CPF_PBI + 1]
            nrm_ap = cpf_sb[:, CPF_NRM:CPF_NRM + NCH]
            w2_idx = {"k": (0, 1), "q": (2, 3)}
            ident_ap = cpr_sb[0:F, 4, :]

            mask_sb = constp.tile([CH, CH], F32)
            make_upper_triangular(nc, mask_sb[:], val=1.0, diag=True)

            # ---- MLPs ----
            feat = {}
            tks = {}
            with (
                tc.tile_pool(name="ph", bufs=4, space=PSUM) as php,
                tc.tile_pool(name="pp", bufs=2, space=PSUM) as ppp,
                tc.tile_pool(name="pv", bufs=2, space=PSUM) as pvp,
            ):
                for _ in range(NWARM):
                    pwarm = php.tile([128, SEG], F32, tag="ph")
                    nc.tensor.matmul(
                        pwarm[:], warm_sb[:, 0:128], warm_sb[:],
                        start=True, stop=True,
                    )

                # h = tanh(x W1 + b1), transposed layout [e, l]
                h_halves = {"k": [], "q": []}
                for name in ("k", "q"):
                    w1sb = w1k_sb if name == "k" else wqv_sb
                    woff = 0
                    for eh in range(2):
                        ph = php.tile([128, SEG], F32, tag="ph")
                        for dc in range(2):
                            nc.tensor.matmul(
                                ph[:],
                                w1sb[:, woff + dc, eh * 128:(eh + 1) * 128],
                                xt_sb[:, dc, :],
                                start=(dc == 0),
                                stop=(dc == 1),
                            )
                        h_sb = hsbp.tile([128, SEG], FR, tag="h")
                        nc.scalar.activation(
                            h_sb[:], ph[:], TANH, bias=b1_ap[name][:, eh:eh + 1]
                        )
                        h_halves[name].append(h_sb)

                # V chunks early: PE gap-filler while ACT runs tanh
                v_sbs = []
                for c in range(NCH):
                    cs = slice(c * CH, (c + 1) * CH)
                    pv = pvp.tile([CH, D], F32, tag="pv")
                    for dc in range(2):
                        nc.tensor.matmul(
                            pv[:],
                            xt_sb[:, dc, cs],
                            wqv_sb[:, 2 + dc, :],
                            start=(dc == 0),
                            stop=(dc == 1),
                        )
                    v_sb = workp.tile([CH, D], FR, tag=f"v{c}")
                    nc.vector.tensor_add(v_sb[:], pv[:], vb_ap)
                    v_sbs.append(v_sb)

                # phase t = tanh(h W2d + b2d), rows doubled [64, SEG]
                for name in ("k", "q"):
                    pp = ppp.tile([F, SEG], F32, tag="pp")
                    for eh in range(2):
                        nc.tensor.matmul(
                            pp[:],
                            cpr_sb[:, w2_idx[name][eh], :],
                            h_halves[name][eh][:],
                            start=(eh == 0),
                            stop=(eh == 1),
                        )
                    tk = workp.tile([F, SEG], F32, tag=f"tk{name}")
                    nc.scalar.activation(tk[:], pp[:], TANH, bias=b2_ap[name])
                    tks[name] = tk

                # keep-warm fillers: PE work covering the tanh/sin chain so
                # HAM stays at 2.4 GHz into the attention phase
                for _ in range(NFILL):
                    pwarm = php.tile([128, SEG], F32, tag="ph")
                    nc.tensor.matmul(
                        pwarm[:], warm_sb[:, 0:128], warm_sb[:],
                        start=True, stop=True,
                    )

                # cos rows need |t|: clear sign bit on DVE (no ACT table load)
                for name in ("k", "q"):
                    tku = tks[name][0:NK, :].bitcast(U32)
                    nc.vector.tensor_scalar(
                        tku, tku, 0x7FFFFFFF, None, op0=AND
                    )
                # feat rows 0-31: sin(pi/2 - pi|t|) = cos; rows 32-63: sin(pi t)
                # emitted per chunk so chunk-0 attention can start while the
                # later sin slices still run
                for name in ("k", "q"):
                    fT = constp.tile([F, SEG], FR, tag=f"feat{name}")
                    feat[name] = fT
                for c in range(NCH):
                    cs = slice(c * CH, (c + 1) * CH)
                    for name in ("k", "q"):
                        nc.scalar.activation(
                            feat[name][:, cs], tks[name][:, cs], SIN,
                            bias=pbi_ap, scale=psc_ap,
                        )
                # qft output is final as soon as sin(q) lands
                nc.scalar.dma_start(qft_d[:, :], feat["q"][:])

            # ---- chunked causal linear attention ----
            with (
                tc.tile_pool(name="po", bufs=2, space=PSUM) as pop,
                tc.tile_pool(name="pat", bufs=3, space=PSUM) as patp,
                tc.tile_pool(name="ps", bufs=1, space=PSUM) as psp,
            ):
                ps_tile = psp.tile([F, D], F32)
                s_sb = None
                for c in range(NCH):
                    cs = slice(c * CH, (c + 1) * CH)
                    # A^T[t,l] = sum_f Kf[t,f] Qf[l,f]; causal-mask (keep l>=t)
                    pat = patp.tile([CH, CH], F32, tag="pat")
                    nc.tensor.matmul(
                        pat[:], feat["k"][:, cs], feat["q"][:, cs],
                        start=True, stop=True,
                    )
                    # Kf chunk transpose immediately after: PE fills the
                    # mask-multiply wait
                    ptr = patp.tile([CH, F], FR, tag="pat")
                    nc.tensor.transpose(ptr[:], feat["k"][:, cs], ident_ap)
                    atm = workp.tile([CH, CH], FR, tag="atm")
                    nc.vector.tensor_mul(atm[:], pat[:], mask_sb[:])
                    kf_sb = workp.tile([CH, F], FR, tag="kf")
                    nc.vector.tensor_copy(kf_sb[:], ptr[:])

                    # O = (A V) [+ Qf S_prev]; scale by 1/sqrt((pos+1)K)
                    po = pop.tile([CH, D], F32, tag="po")
                    nc.tensor.matmul(
                        po[:], atm[:], v_sbs[c][:], start=True, stop=(c == 0)
                    )
                    if c > 0:
                        nc.tensor.matmul(
                            po[:], feat["q"][:, cs], s_sb[:],
                            start=False, stop=True,
                        )
                    # state S += Kf^T V
                    nc.tensor.matmul(
                        ps_tile[:], kf_sb[:], v_sbs[c][:],
                        start=(c == 0), stop=True, skip_group_check=True,
                    )
                    # scale O on ACT (DVE stays on the state chain)
                    o_sb = workp.tile([CH, D], F32, tag="o")
                    nc.scalar.activation(
                        o_sb[:], po[:], mybir.ActivationFunctionType.Copy,
                        scale=nrm_ap[:, c:c + 1],
                    )
                    nc.sync.dma_start(o_d[cs, :], o_sb[:])
                    s_new = workp.tile([F, D], FR, tag="s")
                    nc.vector.tensor_copy(s_new[:], ps_tile[:])
                    s_sb = s_new

                nc.scalar.dma_start(s_d[:, :], s_sb[:])

    nc.compile()
    return nc


def _build_phase_b():
    nc = bacc.Bacc("TRN2", target_bir_lowering=False, debug=False)

    o_in = nc.dram_tensor("o_in", [SEG, D], F32, kind="ExternalInput").ap()
    pk = nc.dram_tensor("pk", [128, SEG + D + NCH], FR, kind="ExternalInput").ap()
    out_d = nc.dram_tensor("out", [SEG, D], F32, kind="ExternalOutput").ap()

    with tile.TileContext(nc) as tc:
        with (
            tc.tile_pool(name="const", bufs=1) as constp,
            tc.tile_pool(name="work", bufs=3) as workp,
            tc.tile_pool(name="pq", bufs=2, space=PSUM) as pqp,
        ):
            pk_sb = constp.tile([128, SEG + D + NCH], FR)
            nc.scalar.dma_start(pk_sb[:], pk[:, :])
            ob_sb = constp.tile([128, NCH, D], F32)
            nc.sync.dma_start(
                ob_sb[:], o_in.rearrange("(c p) d -> p c d", p=CH)
            )
            qft_ap = pk_sb[0:F, 0:SEG]
            spre_ap = pk_sb[0:F, SEG:SEG + D]
            nrm_sb = pk_sb[:, SEG + D:SEG + D + NCH].bitcast(F32)

            for c in range(NCH):
                cs = slice(c * CH, (c + 1) * CH)
                pq = pqp.tile([CH, D], F32, tag="pq")
                nc.tensor.matmul(
                    pq[:], qft_ap[:, cs], spre_ap, start=True, stop=True
                )
                r_sb = workp.tile([CH, D], F32, tag="r")
                nc.vector.scalar_tensor_tensor(
                    r_sb[:], pq[:], nrm_sb[:, c:c + 1], ob_sb[:, c, :],
                    op0=MULT, op1=ADD,
                )
                nc.sync.dma_start(out_d[cs, :], r_sb[:])

    nc.compile()
    return nc


_NC_A = None
_NC_B = None


def _get_ncs():
    global _NC_A, _NC_B
    if _NC_A is None:
        _NC_A = _build_phase_a()
    if _NC_B is None:
        _NC_B = _build_phase_b()
    return _NC_A, _NC_B


def _split_heads(wt):
    # [256, N] row-chunked to [128, 2, N]
    n = wt.shape[1]
    return np.ascontiguousarray(
        wt.reshape(2, 128, n).transpose(1, 0, 2), dtype=np.float32
    )


def _phase_a_in_maps(x, kw1, kb1, kw2, kb2, qw1, qb1, qw2, qb2, vw, vb):
    f32 = np.float32
    w1k = _split_heads(kw1.T)                          # [128, 2, 256]
    wqv = np.concatenate(
        [_split_heads(qw1.T), _split_heads(vw.T)], axis=1
    )                                                  # [128, 4, 256]
    kw2dT = np.vstack([kw2, kw2]).T                    # [256, 64]
    qw2dT = np.vstack([qw2, qw2]).T
    idn = np.zeros((128, F), dtype=f32)
    idn[:F] = np.eye(F, dtype=f32)
    cpr = np.concatenate(
        [_split_heads(kw2dT), _split_heads(qw2dT), idn[:, None, :]], axis=1
    )                                                  # [128, 5, 64]

    cpf = np.zeros((128, CPF_N), dtype=f32)
    cpf[:, CPF_VB:CPF_VB + D] = vb[None, :]
    cpf[:, CPF_B1K:CPF_B1K + 2] = kb1.reshape(2, 128).T
    cpf[:, CPF_B1Q:CPF_B1Q + 2] = qb1.reshape(2, 128).T
    cpf[:F, CPF_B2K] = np.concatenate([kb2, kb2])
    cpf[:F, CPF_B2Q] = np.concatenate([qb2, qb2])
    cpf[:F, CPF_PSC] = np.concatenate([np.full(NK, -PI), np.full(NK, PI)])
    cpf[:F, CPF_PBI] = np.concatenate([np.full(NK, PI / 2), np.zeros(NK)])

    in_maps = []
    for core in range(NCORES):
        b, s = divmod(core, NSEG)
        seg0 = s * SEG
        pos = seg0 + np.arange(SEG, dtype=np.float64) + 1.0
        nrm = (1.0 / np.sqrt(pos * NK)).astype(f32).reshape(NCH, CH).T
        cpf_c = cpf.copy()
        cpf_c[:, CPF_NRM:CPF_NRM + NCH] = nrm
        in_maps.append({
            "w1k": w1k,
            "xt": _split_heads(
                np.ascontiguousarray(x[b, seg0:seg0 + SEG, :].T, dtype=f32)
            ),
            "wqv": wqv,
            "cpr": cpr,
            "cpf": cpf_c,
        })
    return in_maps


def _phase_b_in_maps(res_a):
    f32 = np.float32
    in_maps = []
    for core in range(NCORES):
        b, s = divmod(core, NSEG)
        spre = np.zeros((F, D), dtype=f32)
        for j in range(s):
            spre += res_a[b * NSEG + j]["s"]
        seg0 = s * SEG
        pos = seg0 + np.arange(SEG, dtype=np.float64) + 1.0
        nrm = (1.0 / np.sqrt(pos * NK)).astype(f32).reshape(NCH, CH).T
        pk = np.zeros((128, SEG + D + NCH), dtype=f32)
        pk[:F, 0:SEG] = res_a[core]["qft"]
        pk[:F, SEG:SEG + D] = spre
        pk[:, SEG + D:SEG + D + NCH] = nrm
        in_maps.append({
            "o_in": res_a[core]["o"],
            "pk": pk,
        })
    return in_maps


LAST_RESULTS = []  # [BassKernelResults for phase A, phase B] of the last call


def kernel(**inputs):
    nc_a, nc_b = _get_ncs()
    in_maps_a = _phase_a_in_maps(**{k: np.asarray(v) for k, v in inputs.items()})
    bkr_a = run_bass_kernel_spmd(nc_a, in_maps_a, core_ids=list(range(NCORES)))
    res_a = bkr_a.results
    in_maps_b = _phase_b_in_maps(res_a)
    bkr_b = run_bass_kernel_spmd(nc_b, in_maps_b, core_ids=list(range(NCORES)))
    res_b = bkr_b.results
    LAST_RESULTS[:] = [bkr_a, bkr_b]

    out = np.empty((B, L, D), dtype=np.float32)
    for core in range(NCORES):
        b, s = divmod(core, NSEG)
        out[b, s * SEG:(s + 1) * SEG, :] = res_b[core]["out"]
    return out


# revision 26
# speedup vs baseline: 1.0340x; 1.0340x over previous
"""ContentPhasorStream kernel for 8 Trainium2 NeuronCores.

Math: the reference is causal linear attention with feature map
[cos(phase), sin(phase)] (2K = 64 features):

  retrieved[b,l,d] = sum_{t<=l} v[b,t,d] * sum_k cos(qp[b,l,k] - kp[b,t,k])
                   = Qf[l] @ cumsum_t(Kf[t]^T v[t])      (Qf/Kf = [cos, sin] feats)

so the [B,L,K,D] cumsum never needs to be materialized.  We run a chunked
scan (128-position chunks): per chunk O = mask(Qf Kf^T) V + Qf S with
S += Kf^T V.

Sharding: 8 cores = 2 batches x 4 L-segments of 512.  Phase A computes the
MLPs + intra-segment attention + the segment summary state S_seg [64,256]
per core.  The host then prefix-sums the tiny per-segment states (pure
elementwise adds) and phase B applies the cross-segment correction
out = (Qf @ S_prefix) * norm + O_intra_scaled on-device.

All matmul operand paths use float32r (fp32 bits in the 4x-faster PE
dataflow mode); accumulation stays fp32 in PSUM.

Perf structure (from NTFF traces):
 - inputs packed into 5 large DMAs, issued on two HWDGE rings (sync+scalar),
   first-needed weights first
 - scratch warmup matmuls ramp the PE clock (HAM) during the DMA wait
 - ACT runs tanh x6 then sin x2 (2 LUT loads, no thrash); |t| for the
   cos rows is done on DVE as a sign-bit mask
"""

import math
import sys

import numpy as np

for _p in ("/opt/trn_rl_repo", "/root/.axon_site/_ro/trn_rl_repo"):
    if _p not in sys.path:
        sys.path.insert(0, _p)

# bass_utils imports antenv.axon_hooks when BASS_TRACE is set; provide a
# no-op registry if the image's antenv lacks that module so tracing degrades
# gracefully instead of crashing.
try:
    import antenv.axon_hooks  # noqa: F401
except Exception:
    import types as _types

    _ah = _types.ModuleType("antenv.axon_hooks")
    _ah._HOOK = None

    def _set_hook(h, _m=_ah):
        _m._HOOK = h

    _ah.set_axon_ntff_profile_hook = _set_hook
    _ah.get_axon_ntff_profile_hook = lambda _m=_ah: _m._HOOK
    sys.modules["antenv.axon_hooks"] = _ah

import concourse.bass as bass
import concourse.mybir as mybir
import concourse.tile as tile
from concourse import bacc
from concourse.bass_utils import run_bass_kernel_spmd
from concourse.masks import make_upper_triangular

B, L, D, NK = 2, 2048, 256, 32
NCORES = 8
NSEG = NCORES // B          # 4 segments per batch
SEG = L // NSEG             # 512 positions per core
CH = 128                    # attention chunk
NCH = SEG // CH             # 4 chunks per segment
F = 2 * NK                  # 64 = [cos, sin] feature dim
PI = math.pi
NWARM = 7                   # PE clock-ramp matmuls
NFILL = 6                   # PE keep-warm fillers over the feature-wait gap

F32 = mybir.dt.float32
FR = mybir.dt.float32r
U32 = mybir.dt.uint32
TANH = mybir.ActivationFunctionType.Tanh
SIN = mybir.ActivationFunctionType.Sin
PSUM = bass.MemorySpace.PSUM
AND = mybir.AluOpType.bitwise_and
MULT = mybir.AluOpType.mult
ADD = mybir.AluOpType.add

# cpf (fp32 constant pack) column layout
CPF_VB = 0            # [128, 256] broadcast value bias
CPF_B1K = 256         # [128, 2]
CPF_B1Q = 258         # [128, 2]
CPF_B2K = 260         # [64, 1] doubled key phase bias
CPF_B2Q = 261
CPF_PSC = 262         # [64, 1] sin scale  [-pi x32; +pi x32]
CPF_PBI = 263         # [64, 1] sin bias   [pi/2 x32; 0 x32]
CPF_NRM = 264         # [128, 4] per-chunk 1/sqrt((pos+1)K)
CPF_N = 268


def _build_phase_a():
    nc = bacc.Bacc("TRN2", target_bir_lowering=False, debug=False)

    w1k_d = nc.dram_tensor("w1k", [128, 2, D], FR, kind="ExternalInput").ap()
    xt_d = nc.dram_tensor("xt", [128, 2, SEG], FR, kind="ExternalInput").ap()
    wqv_d = nc.dram_tensor("wqv", [128, 4, D], FR, kind="ExternalInput").ap()
    cpr_d = nc.dram_tensor("cpr", [128, 5, F], FR, kind="ExternalInput").ap()
    cpf_d = nc.dram_tensor("cpf", [128, CPF_N], F32, kind="ExternalInput").ap()

    o_d = nc.dram_tensor("o", [SEG, D], F32, kind="ExternalOutput").ap()
    qft_d = nc.dram_tensor("qft", [F, SEG], FR, kind="ExternalOutput").ap()
    s_d = nc.dram_tensor("s", [F, D], FR, kind="ExternalOutput").ap()

    with tile.TileContext(nc) as tc:
        with (
            tc.tile_pool(name="const", bufs=1) as constp,
            tc.tile_pool(name="hsb", bufs=4) as hsbp,
            tc.tile_pool(name="work", bufs=2) as workp,
        ):
            # ---- PE warmup on scratch data (ramp HAM during DMA wait) ----
            warm_sb = constp.tile([128, SEG], FR)
            nc.vector.memset(warm_sb[:].bitcast(U32), 0)
            # preload the tanh LUT off the critical chain while DMAs are in
            # flight (each tanh<->sin switch reloads the single table slot,
            # so only the first function is worth preloading)
            lutw = constp.tile([1, 1], F32)
            nc.scalar.activation(lutw[:], warm_sb[0:1, 0:1].bitcast(F32), TANH)

            # ---- packed input DMAs, two HWDGE rings, first-needed first ----
            w1k_sb = constp.tile([128, 2, D], FR)
            nc.sync.dma_start(w1k_sb[:], w1k_d[:, :, :])
            xt_sb = constp.tile([128, 2, SEG], FR)
            nc.scalar.dma_start(xt_sb[:, 0, :], xt_d[:, 0, :])
            nc.sync.dma_start(xt_sb[:, 1, :], xt_d[:, 1, :])
            wqv_sb = constp.tile([128, 4, D], FR)
            nc.scalar.dma_start(wqv_sb[:], wqv_d[:, :, :])
            cpf_sb = constp.tile([128, CPF_N], F32)
            nc.sync.dma_start(cpf_sb[:], cpf_d[:, :])
            cpr_sb = constp.tile([128, 5, F], FR)
            nc.scalar.dma_start(cpr_sb[:], cpr_d[:, :, :])

            w1 = {"k": w1k_sb, "q": None}
            vb_ap = cpf_sb[:, CPF_VB:CPF_VB + D]
            b1_ap = {"k": cpf_sb[:, CPF_B1K:CPF_B1K + 2],
                     "q": cpf_sb[:, CPF_B1Q:CPF_B1Q + 2]}
            b2_ap = {"k": cpf_sb[0:F, CPF_B2K:CPF_B2K + 1],
                     "q": cpf_sb[0:F, CPF_B2Q:CPF_B2Q + 1]}
            psc_ap = cpf_sb[0:F, CPF_PSC:CPF_PSC + 1]
            pbi_ap = cpf_sb[0:F, CPF_PBI:CPF_PBI + 1]
            nrm_ap = cpf_sb[:, CPF_NRM:CPF_NRM + NCH]
            w2_idx = {"k": (0, 1), "q": (2, 3)}
            ident_ap = cpr_sb[0:F, 4, :]

            mask_sb = constp.tile([CH, CH], F32)
            make_upper_triangular(nc, mask_sb[:], val=1.0, diag=True)

            # ---- MLPs ----
            feat = {}
            tks = {}
            with (
                tc.tile_pool(name="ph", bufs=4, space=PSUM) as php,
                tc.tile_pool(name="pp", bufs=2, space=PSUM) as ppp,
                tc.tile_pool(name="pv", bufs=2, space=PSUM) as pvp,
            ):
                for _ in range(NWARM):
                    pwarm = php.tile([128, SEG], F32, tag="ph")
                    nc.tensor.matmul(
                        pwarm[:], warm_sb[:, 0:128], warm_sb[:],
                        start=True, stop=True,
                    )

                # h = tanh(x W1 + b1), transposed layout [e, l]
                h_halves = {"k": [], "q": []}
                for name in ("k", "q"):
                    w1sb = w1k_sb if name == "k" else wqv_sb
                    woff = 0
                    for eh in range(2):
                        ph = php.tile([128, SEG], F32, tag="ph")
                        for dc in range(2):
                            nc.tensor.matmul(
                                ph[:],
                                w1sb[:, woff + dc, eh * 128:(eh + 1) * 128],
                                xt_sb[:, dc, :],
                                start=(dc == 0),
                                stop=(dc == 1),
                            )
                        h_sb = hsbp.tile([128, SEG], FR, tag="h")
                        nc.scalar.activation(
                            h_sb[:], ph[:], TANH, bias=b1_ap[name][:, eh:eh + 1]
                        )
                        h_halves[name].append(h_sb)

                # V chunks early: PE gap-filler while ACT runs tanh
                v_sbs = []
                for c in range(NCH):
                    cs = slice(c * CH, (c + 1) * CH)
                    pv = pvp.tile([CH, D], F32, tag="pv")
                    for dc in range(2):
                        nc.tensor.matmul(
                            pv[:],
                            xt_sb[:, dc, cs],
                            wqv_sb[:, 2 + dc, :],
                            start=(dc == 0),
                            stop=(dc == 1),
                        )
                    v_sb = workp.tile([CH, D], FR, tag=f"v{c}")
                    nc.vector.tensor_add(v_sb[:], pv[:], vb_ap)
                    v_sbs.append(v_sb)

                # phase t = tanh(h W2d + b2d), rows doubled [64, SEG]
                for name in ("k", "q"):
                    pp = ppp.tile([F, SEG], F32, tag="pp")
                    for eh in range(2):
                        nc.tensor.matmul(
                            pp[:],
                            cpr_sb[:, w2_idx[name][eh], :],
                            h_halves[name][eh][:],
                            start=(eh == 0),
                            stop=(eh == 1),
                        )
                    tk = workp.tile([F, SEG], F32, tag=f"tk{name}")
                    nc.scalar.activation(tk[:], pp[:], TANH, bias=b2_ap[name])
                    tks[name] = tk

                # keep-warm fillers: PE work covering the tanh/sin chain so
                # HAM stays at 2.4 GHz into the attention phase
                for _ in range(NFILL):
                    pwarm = php.tile([128, SEG], F32, tag="ph")
                    nc.tensor.matmul(
                        pwarm[:], warm_sb[:, 0:128], warm_sb[:],
                        start=True, stop=True,
                    )

                # cos rows need |t|: clear sign bit on DVE (no ACT table load)
                for name in ("k", "q"):
                    tku = tks[name][0:NK, :].bitcast(U32)
                    nc.vector.tensor_scalar(
                        tku, tku, 0x7FFFFFFF, None, op0=AND
                    )
                # feat rows 0-31: sin(pi/2 - pi|t|) = cos; rows 32-63: sin(pi t)
                # emitted per chunk so chunk-0 attention can start while the
                # later sin slices still run
                for name in ("k", "q"):
                    fT = constp.tile([F, SEG], FR, tag=f"feat{name}")
                    feat[name] = fT
                for c in range(NCH):
                    cs = slice(c * CH, (c + 1) * CH)
                    for name in ("k", "q"):
                        nc.scalar.activation(
                            feat[name][:, cs], tks[name][:, cs], SIN,
                            bias=pbi_ap, scale=psc_ap,
                        )
                # qft output is final as soon as sin(q) lands
                nc.scalar.dma_start(qft_d[:, :], feat["q"][:])

            # ---- chunked causal linear attention ----
            with (
                tc.tile_pool(name="po", bufs=2, space=PSUM) as pop,
                tc.tile_pool(name="pat", bufs=3, space=PSUM) as patp,
                tc.tile_pool(name="ps", bufs=1, space=PSUM) as psp,
            ):
                ps_tile = psp.tile([F, D], F32)
                s_sb = None
                for c in range(NCH):
                    cs = slice(c * CH, (c + 1) * CH)
                    # A^T[t,l] = sum_f Kf[t,f] Qf[l,f]; causal-mask (keep l>=t)
                    pat = patp.tile([CH, CH], F32, tag="pat")
                    nc.tensor.matmul(
                        pat[:], feat["k"][:, cs], feat["q"][:, cs],
                        start=True, stop=True,
                    )
                    # Kf chunk transpose immediately after: PE fills the
                    # mask-multiply wait
                    ptr = patp.tile([CH, F], FR, tag="pat")
                    nc.tensor.transpose(ptr[:], feat["k"][:, cs], ident_ap)
                    atm = workp.tile([CH, CH], FR, tag="atm")
                    nc.vector.tensor_mul(atm[:], pat[:], mask_sb[:])
                    kf_sb = workp.tile([CH, F], FR, tag="kf")
                    nc.vector.tensor_copy(kf_sb[:], ptr[:])

                    # O = (A V) [+ Qf S_prev]; scale by 1/sqrt((pos+1)K)
                    po = pop.tile([CH, D], F32, tag="po")
                    nc.tensor.matmul(
                        po[:], atm[:], v_sbs[c][:], start=True, stop=(c == 0)
                    )
                    if c > 0:
                        nc.tensor.matmul(
                            po[:], feat["q"][:, cs], s_sb[:],
                            start=False, stop=True,
                        )
                    # state S += Kf^T V
                    nc.tensor.matmul(
                        ps_tile[:], kf_sb[:], v_sbs[c][:],
                        start=(c == 0), stop=True, skip_group_check=True,
                    )
                    # scale O on ACT (DVE stays on the state chain)
                    o_sb = workp.tile([CH, D], F32, tag="o")
                    nc.scalar.activation(
                        o_sb[:], po[:], mybir.ActivationFunctionType.Copy,
                        scale=nrm_ap[:, c:c + 1],
                    )
                    nc.sync.dma_start(o_d[cs, :], o_sb[:])
                    s_new = workp.tile([F, D], FR, tag="s")
                    nc.vector.tensor_copy(s_new[:], ps_tile[:])
                    s_sb = s_new

                nc.scalar.dma_start(s_d[:, :], s_sb[:])

    nc.compile()
    return nc


def _build_phase_b():
    nc = bacc.Bacc("TRN2", target_bir_lowering=False, debug=False)

    o_in = nc.dram_tensor("o_in", [SEG, D], F32, kind="ExternalInput").ap()
    pk = nc.dram_tensor("pk", [128, SEG + D + NCH], FR, kind="ExternalInput").ap()
    out_d = nc.dram_tensor("out", [SEG, D], F32, kind="ExternalOutput").ap()

    with tile.TileContext(nc) as tc:
        with (
            tc.tile_pool(name="const", bufs=1) as constp,
            tc.tile_pool(name="work", bufs=3) as workp,
            tc.tile_pool(name="pq", bufs=2, space=PSUM) as pqp,
        ):
            pk_sb = constp.tile([128, SEG + D + NCH], FR)
            nc.scalar.dma_start(pk_sb[:], pk[:, :])
            ob_sb = constp.tile([128, NCH, D], F32)
            nc.sync.dma_start(
                ob_sb[:], o_in.rearrange("(c p) d -> p c d", p=CH)
            )
            qft_ap = pk_sb[0:F, 0:SEG]
            spre_ap = pk_sb[0:F, SEG:SEG + D]
            nrm_sb = pk_sb[:, SEG + D:SEG + D + NCH].bitcast(F32)

            for c in range(NCH):
                cs = slice(c * CH, (c + 1) * CH)
                pq = pqp.tile([CH, D], F32, tag="pq")
                nc.tensor.matmul(
                    pq[:], qft_ap[:, cs], spre_ap, start=True, stop=True
                )
                r_sb = workp.tile([CH, D], F32, tag="r")
                nc.vector.scalar_tensor_tensor(
                    r_sb[:], pq[:], nrm_sb[:, c:c + 1], ob_sb[:, c, :],
                    op0=MULT, op1=ADD,
                )
                nc.sync.dma_start(out_d[cs, :], r_sb[:])

    nc.compile()
    return nc


_NC_A = None
_NC_B = None


def _get_ncs():
    global _NC_A, _NC_B
    if _NC_A is None:
        _NC_A = _build_phase_a()
    if _NC_B is None:
        _NC_B = _build_phase_b()
    return _NC_A, _NC_B


def _split_heads(wt):
    # [256, N] row-chunked to [128, 2, N]
    n = wt.shape[1]
    return np.ascontiguousarray(
        wt.reshape(2, 128, n).transpose(1, 0, 2), dtype=np.float32
    )


def _phase_a_in_maps(x, kw1, kb1, kw2, kb2, qw1, qb1, qw2, qb2, vw, vb):
    f32 = np.float32
    w1k = _split_heads(kw1.T)                          # [128, 2, 256]
    wqv = np.concatenate(
        [_split_heads(qw1.T), _split_heads(vw.T)], axis=1
    )                                                  # [128, 4, 256]
    kw2dT = np.vstack([kw2, kw2]).T                    # [256, 64]
    qw2dT = np.vstack([qw2, qw2]).T
    idn = np.zeros((128, F), dtype=f32)
    idn[:F] = np.eye(F, dtype=f32)
    cpr = np.concatenate(
        [_split_heads(kw2dT), _split_heads(qw2dT), idn[:, None, :]], axis=1
    )                                                  # [128, 5, 64]

    cpf = np.zeros((128, CPF_N), dtype=f32)
    cpf[:, CPF_VB:CPF_VB + D] = vb[None, :]
    cpf[:, CPF_B1K:CPF_B1K + 2] = kb1.reshape(2, 128).T
    cpf[:, CPF_B1Q:CPF_B1Q + 2] = qb1.reshape(2, 128).T
    cpf[:F, CPF_B2K] = np.concatenate([kb2, kb2])
    cpf[:F, CPF_B2Q] = np.concatenate([qb2, qb2])
    cpf[:F, CPF_PSC] = np.concatenate([np.full(NK, -PI), np.full(NK, PI)])
    cpf[:F, CPF_PBI] = np.concatenate([np.full(NK, PI / 2), np.zeros(NK)])

    in_maps = []
    for core in range(NCORES):
        b, s = divmod(core, NSEG)
        seg0 = s * SEG
        pos = seg0 + np.arange(SEG, dtype=np.float64) + 1.0
        nrm = (1.0 / np.sqrt(pos * NK)).astype(f32).reshape(NCH, CH).T
        cpf_c = cpf.copy()
        cpf_c[:, CPF_NRM:CPF_NRM + NCH] = nrm
        in_maps.append({
            "w1k": w1k,
            "xt": _split_heads(
                np.ascontiguousarray(x[b, seg0:seg0 + SEG, :].T, dtype=f32)
            ),
            "wqv": wqv,
            "cpr": cpr,
            "cpf": cpf_c,
        })
    return in_maps


def _phase_b_in_maps(res_a):
    f32 = np.float32
    in_maps = []
    for core in range(NCORES):
        b, s = divmod(core, NSEG)
        spre = np.zeros((F, D), dtype=f32)
        for j in range(s):
            spre += res_a[b * NSEG + j]["s"]
        seg0 = s * SEG
        pos = seg0 + np.arange(SEG, dtype=np.float64) + 1.0
        nrm = (1.0 / np.sqrt(pos * NK)).astype(f32).reshape(NCH, CH).T
        pk = np.zeros((128, SEG + D + NCH), dtype=f32)
        pk[:F, 0:SEG] = res_a[core]["qft"]
        pk[:F, SEG:SEG + D] = spre
        pk[:, SEG + D:SEG + D + NCH] = nrm
        in_maps.append({
            "o_in": res_a[core]["o"],
            "pk": pk,
        })
    return in_maps


LAST_RESULTS = []  # [BassKernelResults for phase A, phase B] of the last call


def kernel(**inputs):
    nc_a, nc_b = _get_ncs()
    in_maps_a = _phase_a_in_maps(**{k: np.asarray(v) for k, v in inputs.items()})
    bkr_a = run_bass_kernel_spmd(nc_a, in_maps_a, core_ids=list(range(NCORES)))
    res_a = bkr_a.results
    in_maps_b = _phase_b_in_maps(res_a)
    bkr_b = run_bass_kernel_spmd(nc_b, in_maps_b, core_ids=list(range(NCORES)))
    res_b = bkr_b.results
    LAST_RESULTS[:] = [bkr_a, bkr_b]

    out = np.empty((B, L, D), dtype=np.float32)
    for core in range(NCORES):
        b, s = divmod(core, NSEG)
        out[b, s * SEG:(s + 1) * SEG, :] = res_b[core]["out"]
    return out


# revision 27
# speedup vs baseline: 1.0538x; 1.0192x over previous
"""ContentPhasorStream kernel for 8 Trainium2 NeuronCores.

Math: the reference is causal linear attention with feature map
[cos(phase), sin(phase)] (2K = 64 features):

  retrieved[b,l,d] = sum_{t<=l} v[b,t,d] * sum_k cos(qp[b,l,k] - kp[b,t,k])
                   = Qf[l] @ cumsum_t(Kf[t]^T v[t])      (Qf/Kf = [cos, sin] feats)

so the [B,L,K,D] cumsum never needs to be materialized.  We run a chunked
scan (128-position chunks): per chunk O = mask(Qf Kf^T) V + Qf S with
S += Kf^T V.

Sharding: 8 cores = 2 batches x 4 L-segments of 512.  Phase A computes the
MLPs + intra-segment attention + the segment summary state S_seg [64,256]
per core.  The host then prefix-sums the tiny per-segment states (pure
elementwise adds) and phase B applies the cross-segment correction
out = (Qf @ S_prefix) * norm + O_intra_scaled on-device.

All matmul operand paths use float32r (fp32 bits in the 4x-faster PE
dataflow mode); accumulation stays fp32 in PSUM.

Perf structure (from NTFF traces):
 - inputs packed into 5 large DMAs, issued on two HWDGE rings (sync+scalar),
   first-needed weights first
 - scratch warmup matmuls ramp the PE clock (HAM) during the DMA wait
 - ACT runs tanh x6 then sin x2 (2 LUT loads, no thrash); |t| for the
   cos rows is done on DVE as a sign-bit mask
"""

import math
import sys

import numpy as np

for _p in ("/opt/trn_rl_repo", "/root/.axon_site/_ro/trn_rl_repo"):
    if _p not in sys.path:
        sys.path.insert(0, _p)

# bass_utils imports antenv.axon_hooks when BASS_TRACE is set; provide a
# no-op registry if the image's antenv lacks that module so tracing degrades
# gracefully instead of crashing.
try:
    import antenv.axon_hooks  # noqa: F401
except Exception:
    import types as _types

    _ah = _types.ModuleType("antenv.axon_hooks")
    _ah._HOOK = None

    def _set_hook(h, _m=_ah):
        _m._HOOK = h

    _ah.set_axon_ntff_profile_hook = _set_hook
    _ah.get_axon_ntff_profile_hook = lambda _m=_ah: _m._HOOK
    sys.modules["antenv.axon_hooks"] = _ah

import concourse.bass as bass
import concourse.mybir as mybir
import concourse.tile as tile
from concourse import bacc
from concourse.bass_utils import run_bass_kernel_spmd
from concourse.masks import make_upper_triangular

B, L, D, NK = 2, 2048, 256, 32
NCORES = 8
NSEG = NCORES // B          # 4 segments per batch
SEG = L // NSEG             # 512 positions per core
CH = 128                    # attention chunk
NCH = SEG // CH             # 4 chunks per segment
F = 2 * NK                  # 64 = [cos, sin] feature dim
PI = math.pi
NWARM = 7                   # PE clock-ramp matmuls
NFILL = 6                   # PE keep-warm fillers over the feature-wait gap

F32 = mybir.dt.float32
FR = mybir.dt.float32r
U32 = mybir.dt.uint32
TANH = mybir.ActivationFunctionType.Tanh
SIN = mybir.ActivationFunctionType.Sin
PSUM = bass.MemorySpace.PSUM
AND = mybir.AluOpType.bitwise_and
MULT = mybir.AluOpType.mult
ADD = mybir.AluOpType.add

# cpf (fp32 constant pack) column layout
CPF_VB = 0            # [128, 256] broadcast value bias
CPF_B1K = 256         # [128, 2]
CPF_B1Q = 258         # [128, 2]
CPF_B2K = 260         # [64, 1] doubled key phase bias
CPF_B2Q = 261
CPF_PSC = 262         # [64, 1] sin scale  [-pi x32; +pi x32]
CPF_PBI = 263         # [64, 1] sin bias   [pi/2 x32; 0 x32]
CPF_NRM = 264         # [128, 4] per-chunk 1/sqrt((pos+1)K)
CPF_N = 268


def _build_phase_a():
    nc = bacc.Bacc("TRN2", target_bir_lowering=False, debug=False)

    w1k_d = nc.dram_tensor("w1k", [128, 2, D], FR, kind="ExternalInput").ap()
    xt_d = nc.dram_tensor("xt", [128, 2, SEG], FR, kind="ExternalInput").ap()
    wqv_d = nc.dram_tensor("wqv", [128, 4, D], FR, kind="ExternalInput").ap()
    cpr_d = nc.dram_tensor("cpr", [128, 5, F], FR, kind="ExternalInput").ap()
    cpf_d = nc.dram_tensor("cpf", [128, CPF_N], F32, kind="ExternalInput").ap()

    o_d = nc.dram_tensor("o", [SEG, D], F32, kind="ExternalOutput").ap()
    qft_d = nc.dram_tensor("qft", [F, SEG], FR, kind="ExternalOutput").ap()
    s_d = nc.dram_tensor("s", [F, D], FR, kind="ExternalOutput").ap()

    with tile.TileContext(nc) as tc:
        with (
            tc.tile_pool(name="const", bufs=1) as constp,
            tc.tile_pool(name="hsb", bufs=4) as hsbp,
            tc.tile_pool(name="work", bufs=2) as workp,
        ):
            # ---- PE warmup on scratch data (ramp HAM during DMA wait) ----
            warm_sb = constp.tile([128, SEG], FR)
            nc.vector.memset(warm_sb[:].bitcast(U32), 0)
            # preload the tanh LUT off the critical chain while DMAs are in
            # flight (each tanh<->sin switch reloads the single table slot,
            # so only the first function is worth preloading)
            lutw = constp.tile([1, 1], F32)
            nc.scalar.activation(lutw[:], warm_sb[0:1, 0:1].bitcast(F32), TANH)

            # ---- packed input DMAs, two HWDGE rings, first-needed first ----
            w1k_sb = constp.tile([128, 2, D], FR)
            nc.sync.dma_start(w1k_sb[:], w1k_d[:, :, :])
            xt_sb = constp.tile([128, 2, SEG], FR)
            nc.scalar.dma_start(xt_sb[:, 0, :], xt_d[:, 0, :])
            nc.sync.dma_start(xt_sb[:, 1, :], xt_d[:, 1, :])
            wqv_sb = constp.tile([128, 4, D], FR)
            nc.scalar.dma_start(wqv_sb[:], wqv_d[:, :, :])
            cpf_sb = constp.tile([128, CPF_N], F32)
            nc.sync.dma_start(cpf_sb[:], cpf_d[:, :])
            cpr_sb = constp.tile([128, 5, F], FR)
            nc.scalar.dma_start(cpr_sb[:], cpr_d[:, :, :])

            w1 = {"k": w1k_sb, "q": None}
            vb_ap = cpf_sb[:, CPF_VB:CPF_VB + D]
            b1_ap = {"k": cpf_sb[:, CPF_B1K:CPF_B1K + 2],
                     "q": cpf_sb[:, CPF_B1Q:CPF_B1Q + 2]}
            b2_ap = {"k": cpf_sb[0:F, CPF_B2K:CPF_B2K + 1],
                     "q": cpf_sb[0:F, CPF_B2Q:CPF_B2Q + 1]}
            psc_ap = cpf_sb[0:F, CPF_PSC:CPF_PSC + 1]
            pbi_ap = cpf_sb[0:F, CPF_PBI:CPF_PBI + 1]
            nrm_ap = cpf_sb[:, CPF_NRM:CPF_NRM + NCH]
            w2_idx = {"k": (0, 1), "q": (2, 3)}
            ident_ap = cpr_sb[0:F, 4, :]

            mask_sb = constp.tile([CH, CH], F32)
            make_upper_triangular(nc, mask_sb[:], val=1.0, diag=True)

            # ---- MLPs ----
            feat = {}
            tks = {}
            with (
                tc.tile_pool(name="ph", bufs=4, space=PSUM) as php,
                tc.tile_pool(name="pp", bufs=2, space=PSUM) as ppp,
                tc.tile_pool(name="pv", bufs=2, space=PSUM) as pvp,
            ):
                for _ in range(NWARM):
                    pwarm = php.tile([128, SEG], F32, tag="ph")
                    nc.tensor.matmul(
                        pwarm[:], warm_sb[:, 0:128], warm_sb[:],
                        start=True, stop=True,
                    )

                # h = tanh(x W1 + b1), transposed layout [e, l]
                h_halves = {"k": [], "q": []}
                for name in ("k", "q"):
                    w1sb = w1k_sb if name == "k" else wqv_sb
                    woff = 0
                    for eh in range(2):
                        ph = php.tile([128, SEG], F32, tag="ph")
                        for dc in range(2):
                            nc.tensor.matmul(
                                ph[:],
                                w1sb[:, woff + dc, eh * 128:(eh + 1) * 128],
                                xt_sb[:, dc, :],
                                start=(dc == 0),
                                stop=(dc == 1),
                            )
                        h_sb = hsbp.tile([128, SEG], FR, tag="h")
                        nc.scalar.activation(
                            h_sb[:], ph[:], TANH, bias=b1_ap[name][:, eh:eh + 1]
                        )
                        h_halves[name].append(h_sb)

                # V chunks early: PE gap-filler while ACT runs tanh
                v_sbs = []
                for c in range(NCH):
                    cs = slice(c * CH, (c + 1) * CH)
                    pv = pvp.tile([CH, D], F32, tag="pv")
                    for dc in range(2):
                        nc.tensor.matmul(
                            pv[:],
                            xt_sb[:, dc, cs],
                            wqv_sb[:, 2 + dc, :],
                            start=(dc == 0),
                            stop=(dc == 1),
                        )
                    v_sb = workp.tile([CH, D], FR, tag=f"v{c}")
                    nc.vector.tensor_add(v_sb[:], pv[:], vb_ap)
                    v_sbs.append(v_sb)

                # phase t = tanh(h W2d + b2d), rows doubled [64, SEG]
                for name in ("k", "q"):
                    pp = ppp.tile([F, SEG], F32, tag="pp")
                    for eh in range(2):
                        nc.tensor.matmul(
                            pp[:],
                            cpr_sb[:, w2_idx[name][eh], :],
                            h_halves[name][eh][:],
                            start=(eh == 0),
                            stop=(eh == 1),
                        )
                    tk = workp.tile([F, SEG], F32, tag=f"tk{name}")
                    nc.scalar.activation(tk[:], pp[:], TANH, bias=b2_ap[name])
                    tks[name] = tk

                # keep-warm fillers: PE work covering the tanh/sin chain so
                # HAM stays at 2.4 GHz into the attention phase
                for _ in range(NFILL):
                    pwarm = php.tile([128, SEG], F32, tag="ph")
                    nc.tensor.matmul(
                        pwarm[:], warm_sb[:, 0:128], warm_sb[:],
                        start=True, stop=True,
                    )

                # cos rows need |t|: clear sign bit on DVE (no ACT table load)
                for name in ("k", "q"):
                    tku = tks[name][0:NK, :].bitcast(U32)
                    nc.vector.tensor_scalar(
                        tku, tku, 0x7FFFFFFF, None, op0=AND
                    )
                # feat rows 0-31: sin(pi/2 - pi|t|) = cos; rows 32-63: sin(pi t)
                # emitted per chunk so chunk-0 attention can start while the
                # later sin slices still run
                for name in ("k", "q"):
                    fT = constp.tile([F, SEG], FR, tag=f"feat{name}")
                    feat[name] = fT
                for c in range(NCH):
                    cs = slice(c * CH, (c + 1) * CH)
                    for name in ("k", "q"):
                        nc.scalar.activation(
                            feat[name][:, cs], tks[name][:, cs], SIN,
                            bias=pbi_ap, scale=psc_ap,
                        )
                # qft output is final as soon as sin(q) lands
                nc.scalar.dma_start(qft_d[:, :], feat["q"][:])

            # ---- chunked causal linear attention ----
            with (
                tc.tile_pool(name="po", bufs=2, space=PSUM) as pop,
                tc.tile_pool(name="pat", bufs=4, space=PSUM) as patp,
                tc.tile_pool(name="ps", bufs=1, space=PSUM) as psp,
            ):
                ps_tile = psp.tile([F, D], F32)
                state = {"s": None}

                # software pipeline, 2 chunks deep: AT/transpose of chunk c+1
                # issue before AV/state of chunk c, so the DVE mask/copy
                # latency hides under PE work and HAM stays warm
                def emit_front(c):
                    cs = slice(c * CH, (c + 1) * CH)
                    # A^T[t,l] = sum_f Kf[t,f] Qf[l,f]
                    pat = patp.tile([CH, CH], F32, tag="pat")
                    nc.tensor.matmul(
                        pat[:], feat["k"][:, cs], feat["q"][:, cs],
                        start=True, stop=True,
                    )
                    ptr = patp.tile([CH, F], FR, tag="pat")
                    nc.tensor.transpose(ptr[:], feat["k"][:, cs], ident_ap)
                    atm = workp.tile([CH, CH], FR, tag="atm")
                    nc.vector.tensor_mul(atm[:], pat[:], mask_sb[:])
                    kf_sb = workp.tile([CH, F], FR, tag="kf")
                    nc.vector.tensor_copy(kf_sb[:], ptr[:])
                    return atm, kf_sb

                def emit_back(c, atm, kf_sb):
                    cs = slice(c * CH, (c + 1) * CH)
                    # O = (A V) [+ Qf S_prev]; scale by 1/sqrt((pos+1)K)
                    po = pop.tile([CH, D], F32, tag="po")
                    nc.tensor.matmul(
                        po[:], atm[:], v_sbs[c][:], start=True, stop=(c == 0)
                    )
                    if c > 0:
                        nc.tensor.matmul(
                            po[:], feat["q"][:, cs], state["s"][:],
                            start=False, stop=True,
                        )
                    # state S += Kf^T V
                    nc.tensor.matmul(
                        ps_tile[:], kf_sb[:], v_sbs[c][:],
                        start=(c == 0), stop=True, skip_group_check=True,
                    )
                    # scale O on ACT (DVE stays on the state chain)
                    o_sb = workp.tile([CH, D], F32, tag="o")
                    nc.scalar.activation(
                        o_sb[:], po[:], mybir.ActivationFunctionType.Copy,
                        scale=nrm_ap[:, c:c + 1],
                    )
                    nc.sync.dma_start(o_d[cs, :], o_sb[:])
                    s_new = workp.tile([F, D], FR, tag="s")
                    nc.vector.tensor_copy(s_new[:], ps_tile[:])
                    state["s"] = s_new

                frontc = emit_front(0)
                for c in range(1, NCH):
                    nxt = emit_front(c)
                    emit_back(c - 1, *frontc)
                    frontc = nxt
                emit_back(NCH - 1, *frontc)

                nc.scalar.dma_start(s_d[:, :], state["s"][:])

    nc.compile()
    return nc


def _build_phase_b():
    nc = bacc.Bacc("TRN2", target_bir_lowering=False, debug=False)

    o_in = nc.dram_tensor("o_in", [SEG, D], F32, kind="ExternalInput").ap()
    pk = nc.dram_tensor("pk", [128, SEG + D + NCH], FR, kind="ExternalInput").ap()
    out_d = nc.dram_tensor("out", [SEG, D], F32, kind="ExternalOutput").ap()

    with tile.TileContext(nc) as tc:
        with (
            tc.tile_pool(name="const", bufs=1) as constp,
            tc.tile_pool(name="work", bufs=3) as workp,
            tc.tile_pool(name="pq", bufs=2, space=PSUM) as pqp,
        ):
            pk_sb = constp.tile([128, SEG + D + NCH], FR)
            nc.scalar.dma_start(pk_sb[:], pk[:, :])
            ob_sb = constp.tile([128, NCH, D], F32)
            nc.sync.dma_start(
                ob_sb[:], o_in.rearrange("(c p) d -> p c d", p=CH)
            )
            qft_ap = pk_sb[0:F, 0:SEG]
            spre_ap = pk_sb[0:F, SEG:SEG + D]
            nrm_sb = pk_sb[:, SEG + D:SEG + D + NCH].bitcast(F32)

            for c in range(NCH):
                cs = slice(c * CH, (c + 1) * CH)
                pq = pqp.tile([CH, D], F32, tag="pq")
                nc.tensor.matmul(
                    pq[:], qft_ap[:, cs], spre_ap, start=True, stop=True
                )
                r_sb = workp.tile([CH, D], F32, tag="r")
                nc.vector.scalar_tensor_tensor(
                    r_sb[:], pq[:], nrm_sb[:, c:c + 1], ob_sb[:, c, :],
                    op0=MULT, op1=ADD,
                )
                nc.sync.dma_start(out_d[cs, :], r_sb[:])

    nc.compile()
    return nc


_NC_A = None
_NC_B = None


def _get_ncs():
    global _NC_A, _NC_B
    if _NC_A is None:
        _NC_A = _build_phase_a()
    if _NC_B is None:
        _NC_B = _build_phase_b()
    return _NC_A, _NC_B


def _split_heads(wt):
    # [256, N] row-chunked to [128, 2, N]
    n = wt.shape[1]
    return np.ascontiguousarray(
        wt.reshape(2, 128, n).transpose(1, 0, 2), dtype=np.float32
    )


def _phase_a_in_maps(x, kw1, kb1, kw2, kb2, qw1, qb1, qw2, qb2, vw, vb):
    f32 = np.float32
    w1k = _split_heads(kw1.T)                          # [128, 2, 256]
    wqv = np.concatenate(
        [_split_heads(qw1.T), _split_heads(vw.T)], axis=1
    )                                                  # [128, 4, 256]
    kw2dT = np.vstack([kw2, kw2]).T                    # [256, 64]
    qw2dT = np.vstack([qw2, qw2]).T
    idn = np.zeros((128, F), dtype=f32)
    idn[:F] = np.eye(F, dtype=f32)
    cpr = np.concatenate(
        [_split_heads(kw2dT), _split_heads(qw2dT), idn[:, None, :]], axis=1
    )                                                  # [128, 5, 64]

    cpf = np.zeros((128, CPF_N), dtype=f32)
    cpf[:, CPF_VB:CPF_VB + D] = vb[None, :]
    cpf[:, CPF_B1K:CPF_B1K + 2] = kb1.reshape(2, 128).T
    cpf[:, CPF_B1Q:CPF_B1Q + 2] = qb1.reshape(2, 128).T
    cpf[:F, CPF_B2K] = np.concatenate([kb2, kb2])
    cpf[:F, CPF_B2Q] = np.concatenate([qb2, qb2])
    cpf[:F, CPF_PSC] = np.concatenate([np.full(NK, -PI), np.full(NK, PI)])
    cpf[:F, CPF_PBI] = np.concatenate([np.full(NK, PI / 2), np.zeros(NK)])

    in_maps = []
    for core in range(NCORES):
        b, s = divmod(core, NSEG)
        seg0 = s * SEG
        pos = seg0 + np.arange(SEG, dtype=np.float64) + 1.0
        nrm = (1.0 / np.sqrt(pos * NK)).astype(f32).reshape(NCH, CH).T
        cpf_c = cpf.copy()
        cpf_c[:, CPF_NRM:CPF_NRM + NCH] = nrm
        in_maps.append({
            "w1k": w1k,
            "xt": _split_heads(
                np.ascontiguousarray(x[b, seg0:seg0 + SEG, :].T, dtype=f32)
            ),
            "wqv": wqv,
            "cpr": cpr,
            "cpf": cpf_c,
        })
    return in_maps


def _phase_b_in_maps(res_a):
    f32 = np.float32
    in_maps = []
    for core in range(NCORES):
        b, s = divmod(core, NSEG)
        spre = np.zeros((F, D), dtype=f32)
        for j in range(s):
            spre += res_a[b * NSEG + j]["s"]
        seg0 = s * SEG
        pos = seg0 + np.arange(SEG, dtype=np.float64) + 1.0
        nrm = (1.0 / np.sqrt(pos * NK)).astype(f32).reshape(NCH, CH).T
        pk = np.zeros((128, SEG + D + NCH), dtype=f32)
        pk[:F, 0:SEG] = res_a[core]["qft"]
        pk[:F, SEG:SEG + D] = spre
        pk[:, SEG + D:SEG + D + NCH] = nrm
        in_maps.append({
            "o_in": res_a[core]["o"],
            "pk": pk,
        })
    return in_maps


LAST_RESULTS = []  # [BassKernelResults for phase A, phase B] of the last call


def kernel(**inputs):
    nc_a, nc_b = _get_ncs()
    in_maps_a = _phase_a_in_maps(**{k: np.asarray(v) for k, v in inputs.items()})
    bkr_a = run_bass_kernel_spmd(nc_a, in_maps_a, core_ids=list(range(NCORES)))
    res_a = bkr_a.results
    in_maps_b = _phase_b_in_maps(res_a)
    bkr_b = run_bass_kernel_spmd(nc_b, in_maps_b, core_ids=list(range(NCORES)))
    res_b = bkr_b.results
    LAST_RESULTS[:] = [bkr_a, bkr_b]

    out = np.empty((B, L, D), dtype=np.float32)
    for core in range(NCORES):
        b, s = divmod(core, NSEG)
        out[b, s * SEG:(s + 1) * SEG, :] = res_b[core]["out"]
    return out
